# revision 1
# baseline (speedup 1.0000x reference)
"""Trainium2 Bass kernel for nn_Attention_29497835389298.

The reference module's attention einsum "bhij,bihd->bihd" sums the softmax'd
attention over j while v does not depend on j, so y = v * rowsum(att) == v
(causal softmax rows sum to 1).  The whole module therefore reduces to

    out = x @ (Wv @ Wc) + (bv @ Wc + bc)

Device strategy (8 NeuronCores, no collectives):
  - Output-column sharding: core i owns a 256-column slice of the output.
  - Stage A (on device): M_i = Wv @ Wc[:, shard_i]          (sharded, no redundancy)
  - Stage B (on device): outT_i = M_i.T @ x.T + bias_i      (per-core column slice)
  - Host: layout prep (transposes, bf16 cast, tiny bias fold) and column
    concatenation of the per-core results.

All matmul operands are bf16 (PE full rate, FWL weight loads), accumulation is
fp32 in PSUM.  Measured L2 relative error vs the fp32 reference: ~3e-3.
"""

import numpy as np
import ml_dtypes

import concourse.bass as bass  # noqa: F401  (bass types used via bacc/tile)
import concourse.mybir as mybir
import concourse.tile as tile
from concourse import bacc
from concourse.bass_utils import run_bass_kernel_spmd

P = 128          # partitions
E = 2048         # embed dim
B, S = 4, 2048
T = B * S        # 8192 tokens
NCORES = 8
CS = E // NCORES  # 256 output columns per core
KO = E // P       # 16 k-tiles along any contraction of E
CO = CS // P      # 2 column tiles per core
TCH = 512         # token chunk (moving free dim / PSUM bank width)
NTCH = T // TCH   # 16 chunks

BF16 = mybir.dt.bfloat16
F32 = mybir.dt.float32

# stage-B token chunk schedule (shared by kernel build and host blocking)
CHUNKS = [384, 448] + [512] * 13 + [448, 256]
CH_STARTS = [sum(CHUNKS[:i]) for i in range(len(CHUNKS))]
NWQ = 4
KQ = KO // NWQ

_NC_CACHE = None


def _build():
    nc = bacc.Bacc(
        "TRN2", target_bir_lowering=False, debug=False, num_devices=NCORES
    )

    # DRAM parameters (per-core shards supplied via in_maps)
    wvt = nc.dram_tensor("wvt", [E, E], BF16, kind="ExternalInput").ap()   # Wv.T  [e2, e1]
    # xt/wc/out are HOST-BLOCKED flat buffers: each chunk is stored in its
    # exact SBUF tile layout so every DMA is one fully-linear read/write with
    # 2KiB+ per-partition lines (the [E, T] column-slice pattern only gives
    # 1KiB lines).
    wc = nc.dram_tensor("wc", [E * CS], BF16, kind="ExternalInput").ap()
    xt = nc.dram_tensor("xt", [E * T], BF16, kind="ExternalInput").ap()
    bias = nc.dram_tensor("bias", [P, CO], F32, kind="ExternalInput").ap() # bias[p, co]
    out = nc.dram_tensor("out", [CS * T], BF16, kind="ExternalOutput").ap()

    wvt_r = wvt.rearrange("(ko p) e -> p ko e", p=P)    # [128, 16, 2048]

    with tile.TileContext(nc) as tc:
        with (
            tc.tile_pool(name="const", bufs=1) as cpool,
            tc.tile_pool(name="xin", bufs=5) as xpool,
            tc.tile_pool(name="oout", bufs=3) as opool,
            tc.tile_pool(name="ps", bufs=8, space="PSUM") as pspool,
        ):
            # Stage-A operands loaded as independent k-strips so matmuls can
            # start as soon as the first strips land instead of waiting for
            # the whole 9 MiB.  Wc in 4 chunks so the first strip's matmuls
            # unlock after ~2.5us.
            wc_q = []
            wv_strips = []
            for q in range(NWQ):
                wq = cpool.tile([P, KQ, CS], BF16, tag=f"wcq{q}")
                blk = P * KQ * CS
                nc.sync.dma_start(
                    out=wq[:],
                    in_=wc[q * blk:(q + 1) * blk].rearrange(
                        "(p kq c) -> p kq c", p=P, kq=KQ
                    ),
                )
                wc_q.append(wq)
                for kk in range(KQ):
                    s = cpool.tile([P, E], BF16, tag=f"wv{q}_{kk}")  # 0.5 MiB
                    nc.sync.dma_start(out=s[:], in_=wvt_r[:, q * KQ + kk, :])
                    wv_strips.append(s)
            bias_sb = cpool.tile([P, CO], F32)
            nc.sync.dma_start(out=bias_sb[:], in_=bias[:])
            m_sb = cpool.tile([P, KO, CS], BF16)        # 1 MiB: M_i in [e1_p, e1_o, c]

            # Stage A: M[e1, c] = sum_e2 WvT[e2, e1].T @ Wc[e2, c]
            # k-major over 8 PSUM banks (two mi per bank, disjoint halves):
            # each arriving 0.5 MiB strip immediately unlocks 16 matmuls
            # (~1.7us of PE work > 1.46us strip interarrival), so PE saturates
            # from the first strip.  Single pass: accumulate all 16 k-tiles in
            # PSUM, one [128, 512] eviction per bank at the end.
            # PE warmup: throwaway matmuls while the first strips stream
            # in, so the HAM clock-gate is released (2.4 GHz) by the time
            # real work is ready.
            warm = cpool.tile([P, P], BF16, tag="warm")
            nc.gpsimd.memset(warm[:], 0.0)
            for wi in range(40):
                wps = pspool.tile([P, 2, CS], F32, tag="ps")
                nc.tensor.matmul(
                    wps[:, 0, :P], warm[:], warm[:], start=True, stop=True
                )
            pss = [
                pspool.tile([P, 2, CS], F32, tag="ps", name=f"psA{mp}")
                for mp in range(KO // 2)
            ]
            # NOTE: start=True clears the WHOLE PSUM bank (has_written), so the
            # two half-groups sharing a bank must form ONE group: clear only on
            # the very first matmul; half 1's first write then lands on cleared
            # has_written bits and overwrites, which is exactly what we want.
            for kk in range(KO):
                for mp in range(KO // 2):
                    for half in range(2):
                        mi = 2 * mp + half
                        nc.tensor.matmul(
                            pss[mp][:, half, :],
                            wv_strips[kk][:, mi * P:(mi + 1) * P],
                            wc_q[kk // 4][:, kk % 4, :],
                            start=(kk == 0 and half == 0),
                            stop=(kk == KO - 1 and half == 1),
                        )
            # Evictions 6-7 go to the idle ACT engine so the tail of the
            # serial DVE chain doesn't gate stage B's last m_sb reads.
            for mp in range(KO // 2):
                if mp < 6:
                    nc.vector.tensor_copy(
                        out=m_sb[:, 2 * mp:2 * mp + 2, :], in_=pss[mp][:]
                    )  # f32 -> bf16
                else:
                    nc.scalar.copy(
                        out=m_sb[:, 2 * mp:2 * mp + 2, :], in_=pss[mp][:]
                    )

            # Stage B: outT[c, t] = sum_e1 M[e1, c].T @ xT[e1, t] + bias[c]
            # Output pairs two adjacent chunks per tile so each per-ci DMA has
            # ~2 KiB per-partition lines. NOTE: requires the host out-unblock
            # to use pair-granular blocks.
            PAIRS = [(0, 1, 2, 3), (4, 5, 6, 7), (8, 9, 10, 11),
                     (12, 13, 14, 15), (16,)]
            for grp in PAIRS:
                TBg = sum(CHUNKS[tj] for tj in grp)
                g0 = CH_STARTS[grp[0]]
                o_sb = opool.tile([P, CO, TBg], BF16, tag="o")
                out_ap = out[P * CO * g0:P * CO * (g0 + TBg)].rearrange(
                    "(p co t) -> p co t", p=P, co=CO
                )
                off = 0
                for tj in grp:
                    TB = CHUNKS[tj]
                    t0 = CH_STARTS[tj]
                    x_sb = xpool.tile([P, KO, TB], BF16, tag="x", name=f"x{tj}")
                    nc.sync.dma_start(
                        out=x_sb[:],
                        in_=xt[P * KO * t0:P * KO * (t0 + TB)].rearrange(
                            "(p ko t) -> p ko t", p=P, ko=KO
                        ),
                    )
                    for ci in range(CO):
                        ps = pspool.tile([P, TB], F32, tag="ps")
                        for ki in range(KO):
                            nc.tensor.matmul(
                                ps[:],
                                m_sb[:, ki, ci * P:(ci + 1) * P],
                                x_sb[:, ki, :],
                                start=(ki == 0),
                                stop=(ki == KO - 1),
                            )
                        nc.vector.tensor_tensor(
                            o_sb[:, ci, off:off + TB],
                            ps[:],
                            bias_sb[:, ci, None].to_broadcast([P, TB]),
                            mybir.AluOpType.add,
                        )
                    off += TB
                for ci in range(CO):
                    nc.sync.dma_start(
                        out=out_ap[:, ci, :],
                        in_=o_sb[:, ci, :],
                    )

    nc.compile()
    return nc


def get_nc():
    global _NC_CACHE
    if _NC_CACHE is None:
        _NC_CACHE = _build()
    return _NC_CACHE


def make_in_maps(x, Wv, bv, Wc, bc):
    x = np.asarray(x, dtype=np.float32)
    Wv = np.asarray(Wv, dtype=np.float32)
    bv = np.asarray(bv, dtype=np.float32)
    Wc = np.asarray(Wc, dtype=np.float32)
    bc = np.asarray(bc, dtype=np.float32)

    bf = ml_dtypes.bfloat16
    xt_cols = np.ascontiguousarray(x.reshape(T, E).T).astype(bf)   # [E, T]
    wvt = np.ascontiguousarray(Wv.T).astype(bf)                    # [E, E]

    # block x per chunk into the SBUF tile layout [p][ko][t] (linear DMA)
    xblk = np.empty(E * T, dtype=bf)
    pos = 0
    for t0, TB in zip(CH_STARTS, CHUNKS):
        blk = xt_cols[:, t0:t0 + TB].reshape(KO, P, TB).transpose(1, 0, 2)
        xblk[pos:pos + blk.size] = blk.ravel()
        pos += blk.size

    in_maps = []
    for i in range(NCORES):
        sh = slice(i * CS, (i + 1) * CS)
        wc_sh = np.ascontiguousarray(Wc[:, sh]).astype(bf)         # [E, CS]
        wcblk = np.empty(E * CS, dtype=bf)
        wpos = 0
        for q in range(NWQ):
            blk = wc_sh[q * KQ * P:(q + 1) * KQ * P, :].reshape(
                KQ, P, CS
            ).transpose(1, 0, 2)
            wcblk[wpos:wpos + blk.size] = blk.ravel()
            wpos += blk.size
        bias_full = bv.astype(np.float64) @ Wc[:, sh].astype(np.float64) + bc[sh]
        bias_arr = np.ascontiguousarray(
            bias_full.astype(np.float32).reshape(CO, P).T
        )  # [P, CO]
        in_maps.append({"wvt": wvt, "wc": wcblk, "xt": xblk, "bias": bias_arr})
    return in_maps


def run(in_maps, **kwargs):
    nc = get_nc()
    last_err = None
    for attempt, backoff in enumerate((5.0, 15.0, 30.0, 0.0)):
        try:
            return run_bass_kernel_spmd(nc, in_maps, list(range(NCORES)), **kwargs)
        except Exception as e:  # transient transport/runtime hiccups
            last_err = e
            if backoff:
                import time
                time.sleep(backoff)
    raise last_err


OUT_PAIRS = [(0, 1, 2, 3), (4, 5, 6, 7), (8, 9, 10, 11),
             (12, 13, 14, 15), (16,)]


def assemble(results):
    shards = []
    for i in range(NCORES):
        flat = np.asarray(results[i]["out"])
        outT = np.empty((CO, P, T), dtype=flat.dtype)
        for grp in OUT_PAIRS:
            g0 = CH_STARTS[grp[0]]
            TBg = sum(CHUNKS[tj] for tj in grp)
            blk = flat[P * CO * g0:P * CO * (g0 + TBg)].reshape(P, CO, TBg)
            outT[:, :, g0:g0 + TBg] = blk.transpose(1, 0, 2)
        shards.append(outT.reshape(CS, T))
    full = np.concatenate(shards, axis=0)            # [E, T]
    return np.ascontiguousarray(full.T).astype(np.float32).reshape(B, S, E)


def kernel(x, Wq, bq, Wk, bk, Wv, bv, Wc, bc):
    in_maps = make_in_maps(x, Wv, bv, Wc, bc)
    res = run(in_maps)
    return assemble(res.results)



# revision 2
# speedup vs baseline: 1.1257x; 1.1257x over previous
"""Trainium2 Bass kernel for nn_Attention_29497835389298.

The reference module's attention einsum "bhij,bihd->bihd" sums the softmax'd
attention over j while v does not depend on j, so y = v * rowsum(att) == v
(causal softmax rows sum to 1).  The whole module therefore reduces to

    out = x @ (Wv @ Wc) + (bv @ Wc + bc)

Device strategy (8 NeuronCores, no collectives):
  - Output-column sharding: core i owns a 256-column slice of the output.
  - Stage A (on device, bf16): M64_i = (8 Wv) @ (8 Wc[:, shard_i])
    (M carried at 64x scale so its fp8 quantization lives in e4m3's normal
    range; the stage-B eviction multiplies by 1/64.)
  - Stage B (on device, mixed fp8): outT_i = M_i.T @ x.T + bias_i with the
    contraction's 16 k-tiles split by precision class:
      * a-tiles (ki 0..9):  x as e4m3 hi+lo pair, M as e4m3 hi (+lo pass) --
        DoubleRow fp8 matmuls at 2 k-rows/cycle, quantization error ~1e-3.
      * b-tiles (ki 10..13): x as e4m3 hi only, M compensated via paired
        (Mh,Ml) DoubleRow passes. First-order x error ~2.7e-2/sqrt(16) each.
      * c-tiles (ki 14..15): x and M in e3m4 (4-bit mantissa), plain fp8
        matmul. Error ~1.9e-2/sqrt(16)/sqrt(2)... measured.
    Measured end-to-end rel-L2 error ~1.5e-2 against the fp32 reference
    (gate 2e-2); DMA traffic drops from 45 MiB to 39 MiB per core and PE
    cycles from 136.5us to ~106us.
  - Host: layout prep (transposes, fp8/bf16 casts, tiny bias fold) and
    column concatenation of the per-core results.
"""

import numpy as np
import ml_dtypes

import concourse.bass as bass  # noqa: F401  (bass types used via bacc/tile)
import concourse.mybir as mybir
import concourse.tile as tile
from concourse import bacc
from concourse.bass_utils import run_bass_kernel_spmd

P = 128          # partitions
E = 2048         # embed dim
B, S = 4, 2048
T = B * S        # 8192 tokens
NCORES = 8
CS = E // NCORES  # 256 output columns per core
KO = E // P       # 16 k-tiles along any contraction of E
CO = CS // P      # 2 column tiles per core
TCH = 512         # token chunk (moving free dim / PSUM bank width)

# stage-B k-tile precision classes
NA, NB, NC = 10, 4, 2          # a: e4m3 pair; b: e4m3 hi; c: e3m4
NE4 = NA + NB                   # e4m3 hi slots (ki 0..13)
XSL = NE4 + NA                  # x4 slots per chunk: 14 hi + 10 lo = 24
XL_OFF = NE4                    # xl(k) lives at slot NE4 + k, stride 14

BF16 = mybir.dt.bfloat16
F32 = mybir.dt.float32
FP8E4 = mybir.dt.float8e4
FP8E3 = mybir.dt.float8e3
DR = mybir.MatmulPerfMode.DoubleRow

# stage-B token chunk schedule (shared by kernel build and host blocking)
CHUNKS = [384, 448] + [512] * 13 + [448, 256]
CH_STARTS = [sum(CHUNKS[:i]) for i in range(len(CHUNKS))]
NWQ = 4
KQ = KO // NWQ

OUT_PAIRS = [(0, 1, 2, 3), (4, 5, 6, 7), (8, 9, 10, 11),
             (12, 13, 14, 15), (16,)]

_NC_CACHE = None


def _build():
    nc = bacc.Bacc(
        "TRN2", target_bir_lowering=False, debug=False, num_devices=NCORES
    )

    # DRAM parameters (per-core shards supplied via in_maps).
    # wvt/wc carry the 64x M-scale (host multiplies each factor by 8).
    wvt = nc.dram_tensor("wvt", [E, E], BF16, kind="ExternalInput").ap()
    wc = nc.dram_tensor("wc", [E * CS], BF16, kind="ExternalInput").ap()
    # x4/x3/out are HOST-BLOCKED flat buffers: each chunk is stored in its
    # exact SBUF tile layout so every DMA is one fully-linear read/write.
    x4 = nc.dram_tensor("x4", [P * XSL * T], FP8E4, kind="ExternalInput").ap()
    x3 = nc.dram_tensor("x3", [P * NC * T], FP8E3, kind="ExternalInput").ap()
    bias = nc.dram_tensor("bias", [P, CO], F32, kind="ExternalInput").ap()
    out = nc.dram_tensor("out", [CS * T], BF16, kind="ExternalOutput").ap()

    wvt_r = wvt.rearrange("(ko p) e -> p ko e", p=P)    # [128, 16, 2048]

    with tile.TileContext(nc) as tc:
        with (
            tc.tile_pool(name="const", bufs=1) as cpool,
            tc.tile_pool(name="xin", bufs=5) as xpool,
            tc.tile_pool(name="oout", bufs=3) as opool,
            tc.tile_pool(name="ps", bufs=8, space="PSUM") as pspool,
        ):
            # Stage-A operands loaded as independent k-strips so matmuls can
            # start as soon as the first strips land.
            wc_q = []
            wv_strips = []
            for q in range(NWQ):
                wq = cpool.tile([P, KQ, CS], BF16, tag=f"wcq{q}")
                blk = P * KQ * CS
                nc.sync.dma_start(
                    out=wq[:],
                    in_=wc[q * blk:(q + 1) * blk].rearrange(
                        "(p kq c) -> p kq c", p=P, kq=KQ
                    ),
                )
                wc_q.append(wq)
                for kk in range(KQ):
                    s = cpool.tile([P, E], BF16, tag=f"wv{q}_{kk}")  # 0.5 MiB
                    nc.sync.dma_start(out=s[:], in_=wvt_r[:, q * KQ + kk, :])
                    wv_strips.append(s)
            bias_sb = cpool.tile([P, CO], F32)
            # stage-B weights in fp8: Mh duplicated pairs, Ml, and e3m4 M3
            mh_dup = cpool.tile([P, NE4, 2, CS], FP8E4)
            ml_sb = cpool.tile([P, NE4, CS], FP8E4)
            m3_sb = cpool.tile([P, NC, CS], FP8E3)
            nc.sync.dma_start(out=bias_sb[:], in_=bias[:])

            # Stage A: M64[e1, c] = sum_e2 (8Wv)T[e2, e1].T @ (8Wc)[e2, c]
            # k-major over 8 PSUM banks (two mi per bank, disjoint halves):
            # each arriving 0.5 MiB strip immediately unlocks 16 matmuls.
            pss = [
                pspool.tile([P, 2, CS], F32, tag="ps", name=f"psA{mp}")
                for mp in range(KO // 2)
            ]
            # NOTE: start=True clears the WHOLE PSUM bank (has_written), so the
            # two half-groups sharing a bank must form ONE group: clear only on
            # the very first matmul.
            for kk in range(KO):
                for mp in range(KO // 2):
                    for half in range(2):
                        mi = 2 * mp + half
                        nc.tensor.matmul(
                            pss[mp][:, half, :],
                            wv_strips[kk][:, mi * P:(mi + 1) * P],
                            wc_q[kk // 4][:, kk % 4, :],
                            start=(kk == 0 and half == 0),
                            stop=(kk == KO - 1 and half == 1),
                        )
            # Evictions: quantize M64 into fp8 stage-B weight layouts.
            # Banks 0..6 hold e4m3-class ki pairs; bank 7 holds the c-tiles.
            for mp in range(KO // 2):
                k0 = 2 * mp
                if k0 < NE4:
                    # Mh -> both dup slots (ACT + DVE), Ml = ps - Mh (DVE)
                    nc.scalar.copy(
                        out=mh_dup[:, k0:k0 + 2, 0, :], in_=pss[mp][:]
                    )
                    nc.scalar.copy(
                        out=mh_dup[:, k0:k0 + 2, 1, :], in_=pss[mp][:]
                    )
                    nc.vector.tensor_tensor(
                        out=ml_sb[:, k0:k0 + 2, :],
                        in0=pss[mp][:],
                        in1=mh_dup[:, k0:k0 + 2, 0, :],
                        op=mybir.AluOpType.subtract,
                    )
                else:
                    nc.scalar.copy(out=m3_sb[:, 0:NC, :], in_=pss[mp][:])

            # Stage B: outT[c, t] = (sum_e1 M64[e1, c].T @ xT[e1, t])/64 + b[c]
            for grp in OUT_PAIRS:
                TBg = sum(CHUNKS[tj] for tj in grp)
                g0 = CH_STARTS[grp[0]]
                o_sb = opool.tile([P, CO, TBg], BF16, tag="o")
                out_ap = out[P * CO * g0:P * CO * (g0 + TBg)].rearrange(
                    "(p co t) -> p co t", p=P, co=CO
                )
                off = 0
                for tj in grp:
                    TB = CHUNKS[tj]
                    t0 = CH_STARTS[tj]
                    x4_sb = xpool.tile(
                        [P, XSL, TB], FP8E4, tag="x4", name=f"x4_{tj}"
                    )
                    nc.sync.dma_start(
                        out=x4_sb[:],
                        in_=x4[P * XSL * t0:P * XSL * (t0 + TB)].rearrange(
                            "(p s t) -> p s t", p=P, s=XSL
                        ),
                    )
                    x3_sb = xpool.tile(
                        [P, NC, TB], FP8E3, tag="x3", name=f"x3_{tj}"
                    )
                    nc.sync.dma_start(
                        out=x3_sb[:],
                        in_=x3[P * NC * t0:P * NC * (t0 + TB)].rearrange(
                            "(p s t) -> p s t", p=P, s=NC
                        ),
                    )
                    for ci in range(CO):
                        cs = slice(ci * P, (ci + 1) * P)
                        ps = pspool.tile([P, TB], F32, tag="ps")
                        mms = []
                        # a-tiles pass 1: (xh_k, xl_k) @ (Mh_k, Mh_k)
                        for k in range(NA):
                            mms.append((
                                mh_dup[:, k, :, cs],
                                x4_sb[:, k:k + XL_OFF + 1:XL_OFF, :],
                                DR,
                            ))
                        # b-tiles: (xh_k0, xh_k1) @ (Mh_k0, Mh_k1)
                        for k0 in range(NA, NE4, 2):
                            mms.append((
                                mh_dup[:, k0:k0 + 2, 0, cs],
                                x4_sb[:, k0:k0 + 2, :],
                                DR,
                            ))
                        # Ml pass over all e4 tiles: (xh_k0, xh_k1) @ (Ml_k0, Ml_k1)
                        for k0 in range(0, NE4, 2):
                            mms.append((
                                ml_sb[:, k0:k0 + 2, cs],
                                x4_sb[:, k0:k0 + 2, :],
                                DR,
                            ))
                        # c-tiles: plain e3m4 matmuls
                        for j in range(NC):
                            mms.append((m3_sb[:, j, cs], x3_sb[:, j, :], None))
                        nmm = len(mms)
                        for i, (lhsT, rhs, pm) in enumerate(mms):
                            nc.tensor.matmul(
                                ps[:], lhsT, rhs,
                                start=(i == 0), stop=(i == nmm - 1),
                                perf_mode=pm,
                            )
                        # (ps * 1/64) + bias -> bf16, fused on DVE
                        nc.vector.tensor_scalar(
                            out=o_sb[:, ci, off:off + TB],
                            in0=ps[:],
                            scalar1=1.0 / 64.0,
                            scalar2=bias_sb[:, ci:ci + 1],
                            op0=mybir.AluOpType.mult,
                            op1=mybir.AluOpType.add,
                        )
                    off += TB
                for ci in range(CO):
                    nc.sync.dma_start(
                        out=out_ap[:, ci, :],
                        in_=o_sb[:, ci, :],
                    )

    nc.compile()
    return nc


def get_nc():
    global _NC_CACHE
    if _NC_CACHE is None:
        _NC_CACHE = _build()
    return _NC_CACHE


def make_in_maps(x, Wv, bv, Wc, bc):
    x = np.asarray(x, dtype=np.float32)
    Wv = np.asarray(Wv, dtype=np.float32)
    bv = np.asarray(bv, dtype=np.float32)
    Wc = np.asarray(Wc, dtype=np.float32)
    bc = np.asarray(bc, dtype=np.float32)

    bf = ml_dtypes.bfloat16
    e4 = ml_dtypes.float8_e4m3
    e3 = ml_dtypes.float8_e3m4

    xt_cols = np.ascontiguousarray(x.reshape(T, E).T)              # [E, T] f32
    wvt = np.ascontiguousarray((8.0 * Wv).T).astype(bf)            # [E, E]

    # x fp8 streams: hi for ki 0..13, lo residual for ki 0..9, e3m4 for 14..15
    xe4_rows = NE4 * P
    xh = xt_cols[:xe4_rows].astype(e4)                             # [14*128, T]
    xl = (xt_cols[:NA * P] - xh[:NA * P].astype(np.float32)).astype(e4)
    x3f = xt_cols[xe4_rows:].astype(e3)                            # [2*128, T]

    # block per chunk into SBUF tile layout [p][slot][t] (linear DMA)
    x4blk = np.empty(P * XSL * T, dtype=e4)
    x3blk = np.empty(P * NC * T, dtype=e3)
    p4 = p3 = 0
    for t0, TB in zip(CH_STARTS, CHUNKS):
        hi = xh[:, t0:t0 + TB].reshape(NE4, P, TB)
        lo = xl[:, t0:t0 + TB].reshape(NA, P, TB)
        blk = np.concatenate([hi, lo], axis=0).transpose(1, 0, 2)  # [P,24,TB]
        x4blk[p4:p4 + blk.size] = blk.ravel()
        p4 += blk.size
        b3 = x3f[:, t0:t0 + TB].reshape(NC, P, TB).transpose(1, 0, 2)
        x3blk[p3:p3 + b3.size] = b3.ravel()
        p3 += b3.size

    in_maps = []
    for i in range(NCORES):
        sh = slice(i * CS, (i + 1) * CS)
        wc_sh = np.ascontiguousarray(8.0 * Wc[:, sh]).astype(bf)   # [E, CS]
        wcblk = np.empty(E * CS, dtype=bf)
        wpos = 0
        for q in range(NWQ):
            blk = wc_sh[q * KQ * P:(q + 1) * KQ * P, :].reshape(
                KQ, P, CS
            ).transpose(1, 0, 2)
            wcblk[wpos:wpos + blk.size] = blk.ravel()
            wpos += blk.size
        bias_full = bv.astype(np.float64) @ Wc[:, sh].astype(np.float64) + bc[sh]
        bias_arr = np.ascontiguousarray(
            bias_full.astype(np.float32).reshape(CO, P).T
        )  # [P, CO]
        in_maps.append({
            "wvt": wvt, "wc": wcblk, "x4": x4blk, "x3": x3blk,
            "bias": bias_arr,
        })
    return in_maps


def run(in_maps, **kwargs):
    nc = get_nc()
    last_err = None
    for attempt, backoff in enumerate((5.0, 15.0, 30.0, 0.0)):
        try:
            return run_bass_kernel_spmd(nc, in_maps, list(range(NCORES)), **kwargs)
        except Exception as e:  # transient transport/runtime hiccups
            last_err = e
            if backoff:
                import time
                time.sleep(backoff)
    raise last_err


def assemble(results):
    shards = []
    for i in range(NCORES):
        flat = np.asarray(results[i]["out"])
        outT = np.empty((CO, P, T), dtype=flat.dtype)
        for grp in OUT_PAIRS:
            g0 = CH_STARTS[grp[0]]
            TBg = sum(CHUNKS[tj] for tj in grp)
            blk = flat[P * CO * g0:P * CO * (g0 + TBg)].reshape(P, CO, TBg)
            outT[:, :, g0:g0 + TBg] = blk.transpose(1, 0, 2)
        shards.append(outT.reshape(CS, T))
    full = np.concatenate(shards, axis=0)            # [E, T]
    return np.ascontiguousarray(full.T).astype(np.float32).reshape(B, S, E)


def kernel(x, Wq, bq, Wk, bk, Wv, bv, Wc, bc):
    in_maps = make_in_maps(x, Wv, bv, Wc, bc)
    res = run(in_maps)
    return assemble(res.results)


# revision 5
# speedup vs baseline: 1.1806x; 1.0488x over previous
"""Trainium2 Bass kernel for nn_Attention_29497835389298.

The reference module's attention einsum "bhij,bihd->bihd" sums the softmax'd
attention over j while v does not depend on j, so y = v * rowsum(att) == v
(causal softmax rows sum to 1).  The whole module therefore reduces to

    out = x @ (Wv @ Wc) + (bv @ Wc + bc)

Device strategy (8 NeuronCores, no collectives):
  - Output-column sharding: core i owns a 256-column slice of the output.
  - Stage A (on device, bf16): M64_i = (8 Wv) @ (8 Wc[:, shard_i])
    (M carried at 64x scale so its fp8 quantization lives in e4m3's normal
    range; the stage-B eviction multiplies by 1/64.)
  - Stage B (on device, mixed fp8): outT_i = M_i.T @ x.T + bias_i with the
    contraction's 16 k-tiles split by precision class:
      * a-tiles (ki 0..9):  x as e4m3 hi+lo pair, M as e4m3 hi (+lo pass) --
        DoubleRow fp8 matmuls at 2 k-rows/cycle, quantization error ~1e-3.
      * b-tiles (ki 10..13): x as e4m3 hi only, M compensated via paired
        (Mh,Ml) DoubleRow passes. First-order x error ~2.7e-2/sqrt(16) each.
      * c-tiles (ki 14..15): x and M in e3m4 (4-bit mantissa), plain fp8
        matmul. Error ~1.9e-2/sqrt(16)/sqrt(2)... measured.
    Measured end-to-end rel-L2 error ~1.5e-2 against the fp32 reference
    (gate 2e-2); DMA traffic drops from 45 MiB to 39 MiB per core and PE
    cycles from 136.5us to ~106us.
  - Host: layout prep (transposes, fp8/bf16 casts, tiny bias fold) and
    column concatenation of the per-core results.
"""

import numpy as np
import ml_dtypes

import concourse.bass as bass  # noqa: F401  (bass types used via bacc/tile)
import concourse.mybir as mybir
import concourse.tile as tile
from concourse import bacc
from concourse.bass_utils import run_bass_kernel_spmd

P = 128          # partitions
E = 2048         # embed dim
B, S = 4, 2048
T = B * S        # 8192 tokens
NCORES = 8
CS = E // NCORES  # 256 output columns per core
KO = E // P       # 16 k-tiles along any contraction of E
CO = CS // P      # 2 column tiles per core
TCH = 512         # token chunk (moving free dim / PSUM bank width)

# stage-B k-tile precision classes
NA, NB, NC = 10, 4, 2          # a: e4m3 pair; b: e4m3 hi; c: e3m4
NE4 = NA + NB                   # e4m3 hi slots (ki 0..13)
XSL = NE4 + NA                  # x4 slots per chunk: 14 hi + 10 lo = 24
XL_OFF = NE4                    # xl(k) lives at slot NE4 + k, stride 14

BF16 = mybir.dt.bfloat16
F32 = mybir.dt.float32
FP8E4 = mybir.dt.float8e4
FP8E3 = mybir.dt.float8e3
DR = mybir.MatmulPerfMode.DoubleRow

# stage-B token chunk schedule (shared by kernel build and host blocking)
CHUNKS = [384, 448] + [512] * 13 + [448, 256]
CH_STARTS = [sum(CHUNKS[:i]) for i in range(len(CHUNKS))]
NWQ = 4
KQ = KO // NWQ

OUT_PAIRS = [(0, 1, 2, 3), (4, 5, 6, 7), (8, 9, 10, 11),
             (12, 13, 14, 15), (16,)]

_NC_CACHE = None


def _build():
    nc = bacc.Bacc(
        "TRN2", target_bir_lowering=False, debug=False, num_devices=NCORES
    )

    # DRAM parameters (per-core shards supplied via in_maps).
    # wvt/wc carry the 64x M-scale (host multiplies each factor by 8).
    wvt = nc.dram_tensor("wvt", [E, E], BF16, kind="ExternalInput").ap()
    wc = nc.dram_tensor("wc", [E * CS], BF16, kind="ExternalInput").ap()
    # x4/x3/out are HOST-BLOCKED flat buffers: each chunk is stored in its
    # exact SBUF tile layout so every DMA is one fully-linear read/write.
    x4 = nc.dram_tensor("x4", [P * XSL * T], FP8E4, kind="ExternalInput").ap()
    x3 = nc.dram_tensor("x3", [P * NC * T], FP8E3, kind="ExternalInput").ap()
    bias = nc.dram_tensor("bias", [P, CO], F32, kind="ExternalInput").ap()
    out = nc.dram_tensor("out", [CS * T], BF16, kind="ExternalOutput").ap()

    wvt_r = wvt.rearrange("(ko p) e -> p ko e", p=P)    # [128, 16, 2048]

    with tile.TileContext(nc) as tc:
        with (
            tc.tile_pool(name="const", bufs=1) as cpool,
            tc.tile_pool(name="xin", bufs=8) as xpool,
            tc.tile_pool(name="oout", bufs=4) as opool,
            tc.tile_pool(name="ps", bufs=8, space="PSUM") as pspool,
        ):
            # Stage-A operands loaded as independent k-strips so matmuls can
            # start as soon as the first strips land.
            wc_q = []
            wv_strips = []
            for q in range(NWQ):
                wq = cpool.tile([P, KQ, CS], BF16, tag=f"wcq{q}")
                blk = P * KQ * CS
                nc.sync.dma_start(
                    out=wq[:],
                    in_=wc[q * blk:(q + 1) * blk].rearrange(
                        "(p kq c) -> p kq c", p=P, kq=KQ
                    ),
                )
                wc_q.append(wq)
                for kk in range(KQ):
                    s = cpool.tile([P, E], BF16, tag=f"wv{q}_{kk}")  # 0.5 MiB
                    nc.sync.dma_start(out=s[:], in_=wvt_r[:, q * KQ + kk, :])
                    wv_strips.append(s)
            bias_sb = cpool.tile([P, CO], F32)
            # stage-B weights in fp8: Mh duplicated pairs, Ml, and e3m4 M3
            mh_dup = cpool.tile([P, NE4, 2, CS], FP8E4)
            ml_sb = cpool.tile([P, NE4, CS], FP8E4)
            m3_sb = cpool.tile([P, NC, CS], FP8E3)
            nc.sync.dma_start(out=bias_sb[:], in_=bias[:])

            # Stage A: M64[e1, c] = sum_e2 (8Wv)T[e2, e1].T @ (8Wc)[e2, c]
            # k-major over 8 PSUM banks (two mi per bank, disjoint halves):
            # each arriving 0.5 MiB strip immediately unlocks 16 matmuls.
            pss = [
                pspool.tile([P, 2, CS], F32, tag="ps", name=f"psA{mp}")
                for mp in range(KO // 2)
            ]
            # NOTE: start=True clears the WHOLE PSUM bank (has_written), so the
            # two half-groups sharing a bank must form ONE group: clear only on
            # the very first matmul.
            for kk in range(KO):
                for mp in range(KO // 2):
                    for half in range(2):
                        mi = 2 * mp + half
                        nc.tensor.matmul(
                            pss[mp][:, half, :],
                            wv_strips[kk][:, mi * P:(mi + 1) * P],
                            wc_q[kk // 4][:, kk % 4, :],
                            start=(kk == 0 and half == 0),
                            stop=(kk == KO - 1 and half == 1),
                        )
            # Evictions: quantize M64 into fp8 stage-B weight layouts.
            # Banks 0..6 hold e4m3-class ki pairs; bank 7 holds the c-tiles.
            for mp in range(KO // 2):
                k0 = 2 * mp
                if k0 < NE4:
                    # Mh -> both dup slots (ACT + DVE), Ml = ps - Mh (DVE)
                    nc.scalar.copy(
                        out=mh_dup[:, k0:k0 + 2, 0, :], in_=pss[mp][:]
                    )
                    nc.scalar.copy(
                        out=mh_dup[:, k0:k0 + 2, 1, :], in_=pss[mp][:]
                    )
                    nc.vector.tensor_tensor(
                        out=ml_sb[:, k0:k0 + 2, :],
                        in0=pss[mp][:],
                        in1=mh_dup[:, k0:k0 + 2, 0, :],
                        op=mybir.AluOpType.subtract,
                    )
                else:
                    nc.scalar.copy(out=m3_sb[:, 0:NC, :], in_=pss[mp][:])

            # Stage B: outT[c, t] = (sum_e1 M64[e1, c].T @ xT[e1, t])/64 + b[c]
            # x loads on SP's queue (with the weights, in program order); out
            # stores flush per-chunk on the ACT hwdge queue so their eviction
            # waits never head-of-line-block the x prefetch stream.
            for tj, TB in enumerate(CHUNKS):
                t0 = CH_STARTS[tj]
                x4_sb = xpool.tile(
                    [P, XSL, TB], FP8E4, tag="x4", name=f"x4_{tj}"
                )
                nc.sync.dma_start(
                    out=x4_sb[:],
                    in_=x4[P * XSL * t0:P * XSL * (t0 + TB)].rearrange(
                        "(p s t) -> p s t", p=P, s=XSL
                    ),
                )
                x3_sb = xpool.tile(
                    [P, NC, TB], FP8E3, tag="x3", name=f"x3_{tj}"
                )
                nc.sync.dma_start(
                    out=x3_sb[:],
                    in_=x3[P * NC * t0:P * NC * (t0 + TB)].rearrange(
                        "(p s t) -> p s t", p=P, s=NC
                    ),
                )
                o_sb = opool.tile([P, CO, TB], BF16, tag="o", name=f"o_{tj}")
                for ci in range(CO):
                    cs = slice(ci * P, (ci + 1) * P)
                    ps = pspool.tile([P, TB], F32, tag="ps")
                    mms = []
                    # a-tiles pass 1: (xh_k, xl_k) @ (Mh_k, Mh_k)
                    for k in range(NA):
                        mms.append((
                            mh_dup[:, k, :, cs],
                            x4_sb[:, k:k + XL_OFF + 1:XL_OFF, :],
                            DR,
                        ))
                    # b-tiles: (xh_k0, xh_k1) @ (Mh_k0, Mh_k1)
                    for k0 in range(NA, NE4, 2):
                        mms.append((
                            mh_dup[:, k0:k0 + 2, 0, cs],
                            x4_sb[:, k0:k0 + 2, :],
                            DR,
                        ))
                    # Ml pass over all e4 tiles: (xh_k0, xh_k1) @ (Ml_k0, Ml_k1)
                    for k0 in range(0, NE4, 2):
                        mms.append((
                            ml_sb[:, k0:k0 + 2, cs],
                            x4_sb[:, k0:k0 + 2, :],
                            DR,
                        ))
                    # c-tiles: plain e3m4 matmuls
                    for j in range(NC):
                        mms.append((m3_sb[:, j, cs], x3_sb[:, j, :], None))
                    nmm = len(mms)
                    for i, (lhsT, rhs, pm) in enumerate(mms):
                        nc.tensor.matmul(
                            ps[:], lhsT, rhs,
                            start=(i == 0), stop=(i == nmm - 1),
                            perf_mode=pm,
                        )
                    # (ps * 1/64) + bias -> bf16, fused on DVE
                    nc.vector.tensor_scalar(
                        out=o_sb[:, ci, :],
                        in0=ps[:],
                        scalar1=1.0 / 64.0,
                        scalar2=bias_sb[:, ci:ci + 1],
                        op0=mybir.AluOpType.mult,
                        op1=mybir.AluOpType.add,
                    )
                nc.scalar.dma_start(
                    out=out[P * CO * t0:P * CO * (t0 + TB)].rearrange(
                        "(p co t) -> p co t", p=P, co=CO
                    ),
                    in_=o_sb[:],
                )

    nc.compile()
    return nc


def get_nc():
    global _NC_CACHE
    if _NC_CACHE is None:
        _NC_CACHE = _build()
    return _NC_CACHE


def make_in_maps(x, Wv, bv, Wc, bc):
    x = np.asarray(x, dtype=np.float32)
    Wv = np.asarray(Wv, dtype=np.float32)
    bv = np.asarray(bv, dtype=np.float32)
    Wc = np.asarray(Wc, dtype=np.float32)
    bc = np.asarray(bc, dtype=np.float32)

    bf = ml_dtypes.bfloat16
    e4 = ml_dtypes.float8_e4m3
    e3 = ml_dtypes.float8_e3m4

    xt_cols = np.ascontiguousarray(x.reshape(T, E).T)              # [E, T] f32
    wvt = np.ascontiguousarray((8.0 * Wv).T).astype(bf)            # [E, E]

    # x fp8 streams: hi for ki 0..13, lo residual for ki 0..9, e3m4 for 14..15
    xe4_rows = NE4 * P
    xh = xt_cols[:xe4_rows].astype(e4)                             # [14*128, T]
    xl = (xt_cols[:NA * P] - xh[:NA * P].astype(np.float32)).astype(e4)
    x3f = xt_cols[xe4_rows:].astype(e3)                            # [2*128, T]

    # block per chunk into SBUF tile layout [p][slot][t] (linear DMA)
    x4blk = np.empty(P * XSL * T, dtype=e4)
    x3blk = np.empty(P * NC * T, dtype=e3)
    p4 = p3 = 0
    for t0, TB in zip(CH_STARTS, CHUNKS):
        hi = xh[:, t0:t0 + TB].reshape(NE4, P, TB)
        lo = xl[:, t0:t0 + TB].reshape(NA, P, TB)
        blk = np.concatenate([hi, lo], axis=0).transpose(1, 0, 2)  # [P,24,TB]
        x4blk[p4:p4 + blk.size] = blk.ravel()
        p4 += blk.size
        b3 = x3f[:, t0:t0 + TB].reshape(NC, P, TB).transpose(1, 0, 2)
        x3blk[p3:p3 + b3.size] = b3.ravel()
        p3 += b3.size

    in_maps = []
    for i in range(NCORES):
        sh = slice(i * CS, (i + 1) * CS)
        wc_sh = np.ascontiguousarray(8.0 * Wc[:, sh]).astype(bf)   # [E, CS]
        wcblk = np.empty(E * CS, dtype=bf)
        wpos = 0
        for q in range(NWQ):
            blk = wc_sh[q * KQ * P:(q + 1) * KQ * P, :].reshape(
                KQ, P, CS
            ).transpose(1, 0, 2)
            wcblk[wpos:wpos + blk.size] = blk.ravel()
            wpos += blk.size
        bias_full = bv.astype(np.float64) @ Wc[:, sh].astype(np.float64) + bc[sh]
        bias_arr = np.ascontiguousarray(
            bias_full.astype(np.float32).reshape(CO, P).T
        )  # [P, CO]
        in_maps.append({
            "wvt": wvt, "wc": wcblk, "x4": x4blk, "x3": x3blk,
            "bias": bias_arr,
        })
    return in_maps


def run(in_maps, **kwargs):
    nc = get_nc()
    last_err = None
    for attempt, backoff in enumerate((5.0, 15.0, 30.0, 0.0)):
        try:
            return run_bass_kernel_spmd(nc, in_maps, list(range(NCORES)), **kwargs)
        except Exception as e:  # transient transport/runtime hiccups
            last_err = e
            if backoff:
                import time
                time.sleep(backoff)
    raise last_err


def assemble(results):
    shards = []
    for i in range(NCORES):
        flat = np.asarray(results[i]["out"])
        outT = np.empty((CO, P, T), dtype=flat.dtype)
        for t0, TB in zip(CH_STARTS, CHUNKS):
            blk = flat[P * CO * t0:P * CO * (t0 + TB)].reshape(P, CO, TB)
            outT[:, :, t0:t0 + TB] = blk.transpose(1, 0, 2)
        shards.append(outT.reshape(CS, T))
    full = np.concatenate(shards, axis=0)            # [E, T]
    return np.ascontiguousarray(full.T).astype(np.float32).reshape(B, S, E)


def kernel(x, Wq, bq, Wk, bk, Wv, bv, Wc, bc):
    in_maps = make_in_maps(x, Wv, bv, Wc, bc)
    res = run(in_maps)
    return assemble(res.results)
